# revision 1
# baseline (speedup 1.0000x reference)
"""CornerPooling fused kernel for 8 Trainium2 NeuronCores.

Network (per sample):
  p1 = TopPool(relu(bn(conv3x3(x, w_p1))))      # reverse cummax along H
  p2 = LeftPool(relu(bn(conv3x3(x, w_p2))))     # reverse cummax along W
  t  = bn(conv3x3(p1 + p2, w_pool))
  u  = bn(conv1x1(x, w_c1))
  out = relu(bn(conv3x3(relu(t + u), w_c2)))

Strategy: data-parallel, one sample per core (N=8 over 8 cores). BN folded
into conv weights + per-channel bias on the host; activations cast to bf16
on the host. Convs are per-tap matmuls accumulated in PSUM (channels on
partitions). The two directional reverse-cummax scans run on the vector
engine as single-pass masked TensorTensorScan recurrences:
    state = max(mask * state, value)   (mask=0 restarts a row; relu => >=0)
Branch-1 conv output is produced in W-major (transposed) layout so its scan
along H is contiguous, branch-2 in H-major layout so its scan along W is
contiguous. conv1x1(x) accumulates into the same PSUM tile as conv_pool so
t + u never materializes.
"""

import numpy as np
import ml_dtypes

import concourse.bass as bass
import concourse.mybir as mybir
import concourse.tile as tile
from concourse import bacc
from concourse.bass_utils import run_bass_kernel_spmd

BF16 = mybir.dt.bfloat16
F32 = mybir.dt.float32
NPBF16 = ml_dtypes.bfloat16

N_CORES = 8
C, MID = 256, 128


def _emit(tc, d, H, W):
    nc = tc.nc
    HP, WP = H + 2, W + 2
    SPP = HP * WP
    SP = H * W
    T = 4 * W            # spatial tile: 4 rows x W cols (or 4 cols x H rows)
    NT = SP // T
    CH_T = 16 * H        # top-scan chunk: 16 w-columns, all h
    CH_L = 16 * WP       # left-scan chunk: 16 padded rows
    Relu = mybir.ActivationFunctionType.Relu
    AOp = mybir.AluOpType

    from contextlib import ExitStack
    ctx = ExitStack()
    const = ctx.enter_context(tc.tile_pool(name="const", bufs=1))
    big = ctx.enter_context(tc.tile_pool(name="big", bufs=1))
    psum = ctx.enter_context(tc.tile_pool(name="psum", bufs=8, space="PSUM"))
    ostage = ctx.enter_context(tc.tile_pool(name="ostage", bufs=3))
    xq = ctx.enter_context(tc.tile_pool(name="xq", bufs=4))

    # ---- constants / weights ----
    w1sb = [const.tile([128, 9 * MID], BF16, tag=f"wa{kb}", name=f"w1_{kb}") for kb in range(2)]
    w2sb = [const.tile([128, 9 * MID], BF16, tag=f"w2{kb}", name=f"w2_{kb}") for kb in range(2)]
    wpsb = const.tile([128, 9 * C], BF16, tag="wp", name="wp_sb")
    wc1sb = [const.tile([128, C], BF16, tag=f"wc1{kb}", name=f"wc1_{kb}") for kb in range(2)]
    biassb = const.tile([128, 6], F32, tag="bias", name="bias_sb")
    for kb in range(2):
        nc.sync.dma_start(w1sb[kb][:], d["w1"][kb])
        nc.sync.dma_start(w2sb[kb][:], d["w2"][kb])
        nc.sync.dma_start(wc1sb[kb][:], d["wc1"][kb])
    nc.sync.dma_start(wpsb[:], d["wp"][:])
    nc.sync.dma_start(biassb[:], d["bias"][:])

    mask_l = const.tile([128, CH_L], BF16, tag="mask_l", name="mask_l")
    nc.vector.memset(mask_l[:], 1.0)
    mlv = mask_l[:].rearrange("p (r c) -> p r c", c=WP)
    nc.vector.memset(mlv[:, :, 0:1], 0.0)
    nc.vector.memset(mlv[:, :, WP - 1:WP], 0.0)

    # ---- input (padded, bf16), loaded in row chunks so conv p2 can start
    # after the first chunk instead of the whole 8.5MB ----
    xsb = [big.tile([128, SPP], BF16, tag=f"x{kb}", name=f"x_{kb}") for kb in range(2)]
    rp = max(1, HP // 13)  # rows per DMA chunk
    for r0 in range(0, HP, rp):
        r1 = min(HP, r0 + rp)
        for kb in range(2):
            sl = slice(r0 * WP, r1 * WP)
            nc.sync.dma_start(xsb[kb][:, sl], d["xp"][kb][:, sl])
    xv = [xsb[kb][:].rearrange("p (h w) -> p h w", w=WP) for kb in range(2)]

    a1 = big.tile([128, SP], BF16, tag="a1", name="a1")       # H-major, tight
    a2 = big.tile([128, SPP], BF16, tag="s", name="a2")       # H-major, padded; becomes s
    a2f = a2[:]
    a2v = a2f.rearrange("p (h w) -> p h w", w=WP)
    # zero a2 pad strips (rows 0 / HP-1, cols 0 / WP-1)
    nc.vector.memset(a2v[:, 0:1, :], 0.0)
    nc.vector.memset(a2v[:, HP - 1:HP, :], 0.0)
    nc.vector.memset(a2v[:, :, 0:1], 0.0)
    nc.vector.memset(a2v[:, :, WP - 1:WP], 0.0)

    # ---- phase 1: conv p1 -> a1 (h-major) ----
    a1v = a1[:].rearrange("p (h w) -> p h w", w=W)
    for t in range(NT):
        h0 = 4 * t
        ps = psum.tile([128, T], F32, tag="ps", name="ps")
        for kb in range(2):
            for tap in range(9):
                ky, kx = tap // 3, tap % 3
                rhs = xv[kb][:, h0 + ky:h0 + ky + 4, kx:kx + W]
                nc.tensor.matmul(ps[:], w1sb[kb][:, tap * MID:(tap + 1) * MID], rhs,
                                 start=(kb == 0 and tap == 0), stop=(kb == 1 and tap == 8))
        nc.scalar.activation(a1v[:, h0:h0 + 4, :],
                             ps[:].rearrange("p (a b) -> p a b", b=W),
                             Relu, bias=biassb[:, 0:1])

    # ---- phase 2: conv p2 -> a2 (padded); top-scan columns, left-scan +
    # s-add chunks interleaved (all run on DVE under the PE stream) ----
    a1cols = a1[:].rearrange("p (h w) -> p w h", w=W)  # [p, W, H], h strided
    # top-pool: reverse cummax per column of a1 (strided, in place). Emitted
    # up front (program order!) since the first s-add below reads all of a1;
    # they execute on DVE underneath phase 2's PE stream.
    for w in range(W):
        col = a1cols[:, w, ::-1]
        nc.vector.tensor_tensor_scan(col, col, col, 0.0,
                                     op0=AOp.max, op1=AOp.bypass)
    for t in range(NT):
        h0 = 4 * t
        ps = psum.tile([128, T], F32, tag="ps", name="ps")
        for kb in range(2):
            for tap in range(9):
                ky, kx = tap // 3, tap % 3
                rhs = xv[kb][:, h0 + ky:h0 + ky + 4, kx:kx + W]
                nc.tensor.matmul(ps[:], w2sb[kb][:, tap * MID:(tap + 1) * MID], rhs,
                                 start=(kb == 0 and tap == 0), stop=(kb == 1 and tap == 8))
        nc.scalar.activation(a2v[:, h0 + 1:h0 + 5, 1:1 + W],
                             ps[:].rearrange("p (a b) -> p a b", b=W),
                             Relu, bias=biassb[:, 1:2])
        if t % 4 == 3:
            c = t // 4
            st = (16 * c + 1) * WP
            chunk = a2f[:, st:st + CH_L]
            nc.vector.tensor_tensor_scan(chunk[:, ::-1], mask_l[:], chunk[:, ::-1],
                                         0.0, op0=AOp.mult, op1=AOp.max)
            # s = p1 + p2 for these 16 rows
            dst = a2v[:, 16 * c + 1:16 * c + 17, 1:1 + W]
            nc.vector.tensor_add(dst, dst, a1v[:, 16 * c:16 * c + 16, :])

    # ---- phase 3: conv_pool(s) + conv1x1(x) fused -> r (reuses x slots) ----
    r = [big.tile([128, SPP], BF16, tag=f"x{kb}", name=f"r_{kb}") for kb in range(2)]
    rv = [r[kb][:].rearrange("p (h w) -> p h w", w=WP) for kb in range(2)]
    for kb in range(2):
        nc.vector.memset(rv[kb][:, 0:1, :], 0.0)
        nc.vector.memset(rv[kb][:, HP - 1:HP, :], 0.0)
        nc.vector.memset(rv[kb][:, :, 0:1], 0.0)
        nc.vector.memset(rv[kb][:, :, WP - 1:WP], 0.0)
    xpd = [d["xp"][kb].rearrange("p (h w) -> p h w", w=WP) for kb in range(2)]
    for t in range(NT):
        h0 = 4 * t
        xqt = []
        for kb in range(2):
            q = xq.tile([128, T], BF16, tag=f"xq{kb}", name=f"xq_{kb}")
            nc.sync.dma_start(q[:].rearrange("p (a b) -> p a b", b=W),
                              xpd[kb][:, h0 + 1:h0 + 5, 1:1 + W])
            xqt.append(q)
        for mb in range(2):
            ps = psum.tile([128, T], F32, tag="ps", name="ps")
            for tap in range(9):
                ky, kx = tap // 3, tap % 3
                rhs = a2v[:, h0 + ky:h0 + ky + 4, kx:kx + W]
                nc.tensor.matmul(ps[:], wpsb[:, tap * C + mb * 128:tap * C + mb * 128 + 128],
                                 rhs, start=(tap == 0), stop=False)
            for kb in range(2):
                nc.tensor.matmul(ps[:], wc1sb[kb][:, mb * 128:(mb + 1) * 128], xqt[kb][:],
                                 start=False, stop=(kb == 1))
            nc.scalar.activation(rv[mb][:, h0 + 1:h0 + 5, 1:1 + W],
                                 ps[:].rearrange("p (a b) -> p a b", b=W),
                                 Relu, bias=biassb[:, 2 + mb:3 + mb])

    # ---- phase 4: conv c2 on r -> y (fp32) ----
    wc2sb = [const.tile([128, 9 * C], BF16, tag=f"wa{kb}", name=f"wc2_{kb}") for kb in range(2)]
    for kb in range(2):
        nc.sync.dma_start(wc2sb[kb][:], d["wc2"][kb])
    for t in range(NT):
        h0 = 4 * t
        for mb in range(2):
            ps = psum.tile([128, T], F32, tag="ps", name="ps")
            for kb in range(2):
                for tap in range(9):
                    ky, kx = tap // 3, tap % 3
                    rhs = rv[kb][:, h0 + ky:h0 + ky + 4, kx:kx + W]
                    nc.tensor.matmul(
                        ps[:], wc2sb[kb][:, tap * C + mb * 128:tap * C + mb * 128 + 128],
                        rhs, start=(kb == 0 and tap == 0), stop=(kb == 1 and tap == 8))
            o = ostage.tile([128, T], F32, tag="o", name="o")
            nc.scalar.activation(o[:], ps[:], Relu, bias=biassb[:, 4 + mb:5 + mb])
            nc.sync.dma_start(d["y"][mb][:, t * T:(t + 1) * T], o[:])

    ctx.close()


_MODULE_CACHE = {}


def build_module(H=128, W=128, reps=1):
    key = (H, W, reps)
    if key in _MODULE_CACHE:
        return _MODULE_CACHE[key]
    SPP = (H + 2) * (W + 2)
    nc = bacc.Bacc("TRN2", debug=False)
    d = {}
    d["xp"] = nc.dram_tensor("xp", [2, 128, SPP], BF16, kind="ExternalInput").ap()
    d["w1"] = nc.dram_tensor("w1", [2, 128, 9 * MID], BF16, kind="ExternalInput").ap()
    d["w2"] = nc.dram_tensor("w2", [2, 128, 9 * MID], BF16, kind="ExternalInput").ap()
    d["wp"] = nc.dram_tensor("wp", [128, 9 * C], BF16, kind="ExternalInput").ap()
    d["wc1"] = nc.dram_tensor("wc1", [2, 128, C], BF16, kind="ExternalInput").ap()
    d["wc2"] = nc.dram_tensor("wc2", [2, 128, 9 * C], BF16, kind="ExternalInput").ap()
    d["bias"] = nc.dram_tensor("bias", [128, 6], F32, kind="ExternalInput").ap()
    d["y"] = nc.dram_tensor("y", [2, 128, H * W], F32, kind="ExternalOutput").ap()
    with tile.TileContext(nc) as tc:
        for _ in range(reps):
            _emit(tc, d, H, W)
    nc.compile()
    _MODULE_CACHE[key] = nc
    return nc


def _fold(w, g, b, m, v, eps=1e-5):
    inv = g / np.sqrt(v + eps)
    return (w * inv[:, None, None, None]).astype(np.float32), (b - m * inv).astype(np.float32)


def _lhsT(w):
    """[O, I, kh, kw] -> [I//128, 128, 9*O] bf16, index tap*O + o."""
    O, I = w.shape[0], w.shape[1]
    t = np.ascontiguousarray(np.transpose(w, (1, 2, 3, 0)).reshape(I, -1))
    return t.reshape(I // 128, 128, -1).astype(NPBF16)


def prep_host(inputs, H=128, W=128):
    """Fold BN, reshape weights, pad+cast x. Returns (shared, per_core_xp)."""
    HP, WP = H + 2, W + 2
    w1f, b1 = _fold(inputs["w_p1"], inputs["g_p1"], inputs["b_p1"], inputs["m_p1"], inputs["v_p1"])
    w2f, b2 = _fold(inputs["w_p2"], inputs["g_p2"], inputs["b_p2"], inputs["m_p2"], inputs["v_p2"])
    wpf, bp = _fold(inputs["w_pool"], inputs["g_pool"], inputs["b_pool"], inputs["m_pool"], inputs["v_pool"])
    wc1f, bc1 = _fold(inputs["w_c1"], inputs["g_c1"], inputs["b_c1"], inputs["m_c1"], inputs["v_c1"])
    wc2f, bc2 = _fold(inputs["w_c2"], inputs["g_c2"], inputs["b_c2"], inputs["m_c2"], inputs["v_c2"])
    br = bp + bc1

    bias = np.zeros((128, 6), np.float32)
    bias[:, 0] = b1
    bias[:, 1] = b2
    bias[:, 2] = br[:128]
    bias[:, 3] = br[128:]
    bias[:, 4] = bc2[:128]
    bias[:, 5] = bc2[128:]

    shared = {
        "w1": _lhsT(w1f), "w2": _lhsT(w2f),
        "wp": _lhsT(wpf)[0], "wc1": _lhsT(wc1f), "wc2": _lhsT(wc2f),
        "bias": bias,
    }

    x = np.asarray(inputs["in_feature"], np.float32)  # [N, 256, H, W]
    N = x.shape[0]
    xp = np.zeros((N, 2, 128, HP, WP), NPBF16)
    xp[:, :, :, 1:1 + H, 1:1 + W] = x.reshape(N, 2, 128, H, W).astype(NPBF16)
    xp = xp.reshape(N, 2, 128, HP * WP)
    return shared, xp


def kernel(**inputs):
    H = W = 128
    nc = build_module(H, W)
    shared, xp = prep_host(inputs, H, W)
    n = xp.shape[0]
    in_maps = [dict(shared, xp=np.ascontiguousarray(xp[i])) for i in range(n)]
    res = run_bass_kernel_spmd(nc, in_maps, core_ids=list(range(n)))
    outs = [r["y"].reshape(C, H, W) for r in res.results]
    return np.stack(outs).astype(np.float32)



# revision 6
# speedup vs baseline: 1.0084x; 1.0084x over previous
"""CornerPooling fused kernel for 8 Trainium2 NeuronCores.

Network (per sample):
  p1 = TopPool(relu(bn(conv3x3(x, w_p1))))      # reverse cummax along H
  p2 = LeftPool(relu(bn(conv3x3(x, w_p2))))     # reverse cummax along W
  t  = bn(conv3x3(p1 + p2, w_pool))
  u  = bn(conv1x1(x, w_c1))
  out = relu(bn(conv3x3(relu(t + u), w_c2)))

Strategy: data-parallel, one sample per core (N=8 over 8 cores). BN folded
into conv weights + per-channel bias on the host; activations cast to bf16
on the host. Convs are per-tap matmuls accumulated in PSUM (channels on
partitions). The two directional reverse-cummax scans run on the vector
engine as single-pass masked TensorTensorScan recurrences:
    state = max(mask * state, value)   (mask=0 restarts a row; relu => >=0)
Branch-1 conv output is produced in W-major (transposed) layout so its scan
along H is contiguous, branch-2 in H-major layout so its scan along W is
contiguous. conv1x1(x) accumulates into the same PSUM tile as conv_pool so
t + u never materializes.
"""

import numpy as np
import ml_dtypes

import concourse.bass as bass
import concourse.mybir as mybir
import concourse.tile as tile
from concourse import bacc
from concourse.bass_utils import run_bass_kernel_spmd

BF16 = mybir.dt.bfloat16
F32 = mybir.dt.float32
NPBF16 = ml_dtypes.bfloat16

N_CORES = 8
C, MID = 256, 128


def _emit(tc, d, H, W):
    nc = tc.nc
    HP, WP = H + 2, W + 2
    SPP = HP * WP
    SP = H * W
    T = 4 * W            # spatial tile: 4 rows x W cols (or 4 cols x H rows)
    NT = SP // T
    CH_T = 16 * H        # top-scan chunk: 16 w-columns, all h
    CH_L = 16 * WP       # left-scan chunk: 16 padded rows
    Relu = mybir.ActivationFunctionType.Relu
    AOp = mybir.AluOpType

    from contextlib import ExitStack
    ctx = ExitStack()
    const = ctx.enter_context(tc.tile_pool(name="const", bufs=1))
    big = ctx.enter_context(tc.tile_pool(name="big", bufs=1))
    psum = ctx.enter_context(tc.tile_pool(name="psum", bufs=8, space="PSUM"))
    ostage = ctx.enter_context(tc.tile_pool(name="ostage", bufs=3))
    xq = ctx.enter_context(tc.tile_pool(name="xq", bufs=4))

    # ---- constants / weights ----
    # DMA ordering is critical-path-aware: the sync HWDGE ring carries the
    # weights (w1[0] first — it gates the first matmul); the scalar HWDGE
    # ring carries the x chunks in parallel, smallest-first.
    w1sb = [const.tile([128, 9 * MID], BF16, tag=f"wa{kb}", name=f"w1_{kb}") for kb in range(2)]
    w2sb = [const.tile([128, 9 * MID], BF16, tag=f"w2{kb}", name=f"w2_{kb}") for kb in range(2)]
    wpsb = const.tile([128, 9 * C], BF16, tag="wp", name="wp_sb")
    wc1sb = [const.tile([128, C], BF16, tag=f"wc1{kb}", name=f"wc1_{kb}") for kb in range(2)]
    biassb = const.tile([128, 6], F32, tag="bias", name="bias_sb")
    nc.sync.dma_start(w1sb[0][:], d["w1"][0])
    nc.sync.dma_start(w1sb[1][:], d["w1"][1])
    nc.sync.dma_start(biassb[:], d["bias"][:])
    for kb in range(2):
        nc.sync.dma_start(w2sb[kb][:], d["w2"][kb])
    nc.sync.dma_start(wpsb[:], d["wp"][:])
    for kb in range(2):
        nc.sync.dma_start(wc1sb[kb][:], d["wc1"][kb])

    mask_l = const.tile([128, CH_L], BF16, tag="mask_l", name="mask_l")
    nc.vector.memset(mask_l[:], 1.0)
    mlv = mask_l[:].rearrange("p (r c) -> p r c", c=WP)
    nc.vector.memset(mlv[:, :, 0:1], 0.0)
    nc.vector.memset(mlv[:, :, WP - 1:WP], 0.0)

    # ---- input (padded, bf16), loaded in row chunks so conv p1 can start
    # after the first small chunk instead of the whole 8.5MB. Runs on the
    # scalar HWDGE ring, parallel to the weight loads on the sync ring. ----
    xsb = [big.tile([128, SPP], BF16, tag=f"x{kb}", name=f"x_{kb}") for kb in range(2)]
    first = 6  # rows needed by tile 0 (h0=0, ky<=2, 4 rows)
    for kb in range(2):
        nc.scalar.dma_start(xsb[kb][:, 0:first * WP], d["xp"][kb][:, 0:first * WP])
    rp = max(1, HP // 13)  # rows per DMA chunk
    for r0 in range(first, HP, rp):
        r1 = min(HP, r0 + rp)
        for kb in range(2):
            sl = slice(r0 * WP, r1 * WP)
            nc.scalar.dma_start(xsb[kb][:, sl], d["xp"][kb][:, sl])
    xv = [xsb[kb][:].rearrange("p (h w) -> p h w", w=WP) for kb in range(2)]

    a1 = big.tile([128, SP], BF16, tag="a1", name="a1")       # H-major, tight
    a2 = big.tile([128, SPP], BF16, tag="s", name="a2")       # H-major, padded; becomes s
    a2f = a2[:]
    a2v = a2f.rearrange("p (h w) -> p h w", w=WP)
    # zero a2 pad strips (rows 0 / HP-1, cols 0 / WP-1)
    nc.vector.memset(a2v[:, 0:1, :], 0.0)
    nc.vector.memset(a2v[:, HP - 1:HP, :], 0.0)
    nc.vector.memset(a2v[:, :, 0:1], 0.0)
    nc.vector.memset(a2v[:, :, WP - 1:WP], 0.0)

    # ---- phase 1: conv p1 -> a1 (h-major) ----
    a1v = a1[:].rearrange("p (h w) -> p h w", w=W)
    for t in range(NT):
        h0 = 4 * t
        ps = psum.tile([128, T], F32, tag="ps", name="ps")
        for kb in range(2):
            for tap in range(9):
                ky, kx = tap // 3, tap % 3
                rhs = xv[kb][:, h0 + ky:h0 + ky + 4, kx:kx + W]
                nc.tensor.matmul(ps[:], w1sb[kb][:, tap * MID:(tap + 1) * MID], rhs,
                                 start=(kb == 0 and tap == 0), stop=(kb == 1 and tap == 8))
        nc.scalar.activation(a1v[:, h0:h0 + 4, :],
                             ps[:].rearrange("p (a b) -> p a b", b=W),
                             Relu, bias=biassb[:, 0:1])

    # wc2 reuses w1's SBUF slots (w1's last read is the end of phase 1); its
    # DMA waits on that dep at the idle sync ring and lands during phase 2.
    wc2sb = [const.tile([128, 9 * C], BF16, tag=f"wa{kb}", name=f"wc2_{kb}") for kb in range(2)]
    for kb in range(2):
        nc.sync.dma_start(wc2sb[kb][:], d["wc2"][kb])

    # ---- phase 2: conv p2 -> a2 (padded); top-scan columns, left-scan +
    # s-add chunks interleaved (all run on DVE under the PE stream) ----
    a1cols = a1[:].rearrange("p (h w) -> p w h", w=W)  # [p, W, H], h strided
    # top-pool: reverse cummax per column of a1 (strided, in place). Emitted
    # up front (program order!) since the first s-add below reads all of a1;
    # they execute on DVE underneath phase 2's PE stream.
    for w in range(W):
        col = a1cols[:, w, ::-1]
        nc.vector.tensor_tensor_scan(col, col, col, 0.0,
                                     op0=AOp.max, op1=AOp.bypass)
    for t in range(NT):
        h0 = 4 * t
        ps = psum.tile([128, T], F32, tag="ps", name="ps")
        for kb in range(2):
            for tap in range(9):
                ky, kx = tap // 3, tap % 3
                rhs = xv[kb][:, h0 + ky:h0 + ky + 4, kx:kx + W]
                nc.tensor.matmul(ps[:], w2sb[kb][:, tap * MID:(tap + 1) * MID], rhs,
                                 start=(kb == 0 and tap == 0), stop=(kb == 1 and tap == 8))
        nc.scalar.activation(a2v[:, h0 + 1:h0 + 5, 1:1 + W],
                             ps[:].rearrange("p (a b) -> p a b", b=W),
                             Relu, bias=biassb[:, 1:2])
        if t % 4 == 3:
            c = t // 4
            st = (16 * c + 1) * WP
            chunk = a2f[:, st:st + CH_L]
            nc.vector.tensor_tensor_scan(chunk[:, ::-1], mask_l[:], chunk[:, ::-1],
                                         0.0, op0=AOp.mult, op1=AOp.max)
            # s = p1 + p2 for these 16 rows
            dst = a2v[:, 16 * c + 1:16 * c + 17, 1:1 + W]
            nc.vector.tensor_add(dst, dst, a1v[:, 16 * c:16 * c + 16, :])

    # ---- phase 3: conv_pool(s) + conv1x1(x) fused -> r (reuses x slots) ----
    r = [big.tile([128, SPP], BF16, tag=f"x{kb}", name=f"r_{kb}") for kb in range(2)]
    rv = [r[kb][:].rearrange("p (h w) -> p h w", w=WP) for kb in range(2)]
    for kb in range(2):
        nc.vector.memset(rv[kb][:, 0:1, :], 0.0)
        nc.vector.memset(rv[kb][:, HP - 1:HP, :], 0.0)
        nc.vector.memset(rv[kb][:, :, 0:1], 0.0)
        nc.vector.memset(rv[kb][:, :, WP - 1:WP], 0.0)
    xpd = [d["xp"][kb].rearrange("p (h w) -> p h w", w=WP) for kb in range(2)]
    for t in range(NT):
        h0 = 4 * t
        xqt = []
        for kb in range(2):
            q = xq.tile([128, T], BF16, tag=f"xq{kb}", name=f"xq_{kb}")
            nc.scalar.dma_start(q[:].rearrange("p (a b) -> p a b", b=W),
                                xpd[kb][:, h0 + 1:h0 + 5, 1:1 + W])
            xqt.append(q)
        for mb in range(2):
            ps = psum.tile([128, T], F32, tag="ps", name="ps")
            for tap in range(9):
                ky, kx = tap // 3, tap % 3
                rhs = a2v[:, h0 + ky:h0 + ky + 4, kx:kx + W]
                nc.tensor.matmul(ps[:], wpsb[:, tap * C + mb * 128:tap * C + mb * 128 + 128],
                                 rhs, start=(tap == 0), stop=False)
            for kb in range(2):
                nc.tensor.matmul(ps[:], wc1sb[kb][:, mb * 128:(mb + 1) * 128], xqt[kb][:],
                                 start=False, stop=(kb == 1))
            nc.scalar.activation(rv[mb][:, h0 + 1:h0 + 5, 1:1 + W],
                                 ps[:].rearrange("p (a b) -> p a b", b=W),
                                 Relu, bias=biassb[:, 2 + mb:3 + mb])

    # ---- phase 4: conv c2 on r -> y (fp32). The last 4-row tile is split
    # into two 2-row sub-tiles so the final output DMA is small; output DMAs
    # alternate between the sync (mb=0) and scalar (mb=1) HWDGE rings. ----
    sub = [(4 * t, 4) for t in range(NT - 1)] + [(4 * NT - 4, 2), (4 * NT - 2, 2)]
    for h0, nr in sub:
        Ts = nr * W
        for mb in range(2):
            ps = psum.tile([128, T], F32, tag="ps", name="ps")
            for kb in range(2):
                for tap in range(9):
                    ky, kx = tap // 3, tap % 3
                    rhs = rv[kb][:, h0 + ky:h0 + ky + nr, kx:kx + W]
                    nc.tensor.matmul(
                        ps[:, 0:Ts], wc2sb[kb][:, tap * C + mb * 128:tap * C + mb * 128 + 128],
                        rhs, start=(kb == 0 and tap == 0), stop=(kb == 1 and tap == 8))
            o = ostage.tile([128, T], F32, tag="o", name="o")
            nc.scalar.activation(o[:, 0:Ts], ps[:, 0:Ts], Relu, bias=biassb[:, 4 + mb:5 + mb])
            eng = nc.sync if mb == 0 else nc.scalar
            eng.dma_start(d["y"][mb][:, h0 * W:h0 * W + Ts], o[:, 0:Ts])

    ctx.close()


_MODULE_CACHE = {}


def build_module(H=128, W=128, reps=1):
    key = (H, W, reps)
    if key in _MODULE_CACHE:
        return _MODULE_CACHE[key]
    SPP = (H + 2) * (W + 2)
    nc = bacc.Bacc("TRN2", debug=False)
    d = {}
    d["xp"] = nc.dram_tensor("xp", [2, 128, SPP], BF16, kind="ExternalInput").ap()
    d["w1"] = nc.dram_tensor("w1", [2, 128, 9 * MID], BF16, kind="ExternalInput").ap()
    d["w2"] = nc.dram_tensor("w2", [2, 128, 9 * MID], BF16, kind="ExternalInput").ap()
    d["wp"] = nc.dram_tensor("wp", [128, 9 * C], BF16, kind="ExternalInput").ap()
    d["wc1"] = nc.dram_tensor("wc1", [2, 128, C], BF16, kind="ExternalInput").ap()
    d["wc2"] = nc.dram_tensor("wc2", [2, 128, 9 * C], BF16, kind="ExternalInput").ap()
    d["bias"] = nc.dram_tensor("bias", [128, 6], F32, kind="ExternalInput").ap()
    d["y"] = nc.dram_tensor("y", [2, 128, H * W], F32, kind="ExternalOutput").ap()
    with tile.TileContext(nc) as tc:
        for _ in range(reps):
            _emit(tc, d, H, W)
    nc.compile()
    _MODULE_CACHE[key] = nc
    return nc


def _fold(w, g, b, m, v, eps=1e-5):
    inv = g / np.sqrt(v + eps)
    return (w * inv[:, None, None, None]).astype(np.float32), (b - m * inv).astype(np.float32)


def _lhsT(w):
    """[O, I, kh, kw] -> [I//128, 128, 9*O] bf16, index tap*O + o."""
    O, I = w.shape[0], w.shape[1]
    t = np.ascontiguousarray(np.transpose(w, (1, 2, 3, 0)).reshape(I, -1))
    return t.reshape(I // 128, 128, -1).astype(NPBF16)


def prep_host(inputs, H=128, W=128):
    """Fold BN, reshape weights, pad+cast x. Returns (shared, per_core_xp)."""
    HP, WP = H + 2, W + 2
    w1f, b1 = _fold(inputs["w_p1"], inputs["g_p1"], inputs["b_p1"], inputs["m_p1"], inputs["v_p1"])
    w2f, b2 = _fold(inputs["w_p2"], inputs["g_p2"], inputs["b_p2"], inputs["m_p2"], inputs["v_p2"])
    wpf, bp = _fold(inputs["w_pool"], inputs["g_pool"], inputs["b_pool"], inputs["m_pool"], inputs["v_pool"])
    wc1f, bc1 = _fold(inputs["w_c1"], inputs["g_c1"], inputs["b_c1"], inputs["m_c1"], inputs["v_c1"])
    wc2f, bc2 = _fold(inputs["w_c2"], inputs["g_c2"], inputs["b_c2"], inputs["m_c2"], inputs["v_c2"])
    br = bp + bc1

    bias = np.zeros((128, 6), np.float32)
    bias[:, 0] = b1
    bias[:, 1] = b2
    bias[:, 2] = br[:128]
    bias[:, 3] = br[128:]
    bias[:, 4] = bc2[:128]
    bias[:, 5] = bc2[128:]

    shared = {
        "w1": _lhsT(w1f), "w2": _lhsT(w2f),
        "wp": _lhsT(wpf)[0], "wc1": _lhsT(wc1f), "wc2": _lhsT(wc2f),
        "bias": bias,
    }

    x = np.asarray(inputs["in_feature"], np.float32)  # [N, 256, H, W]
    N = x.shape[0]
    xp = np.zeros((N, 2, 128, HP, WP), NPBF16)
    xp[:, :, :, 1:1 + H, 1:1 + W] = x.reshape(N, 2, 128, H, W).astype(NPBF16)
    xp = xp.reshape(N, 2, 128, HP * WP)
    return shared, xp


def kernel(**inputs):
    H = W = 128
    nc = build_module(H, W)
    shared, xp = prep_host(inputs, H, W)
    n = xp.shape[0]
    in_maps = [dict(shared, xp=np.ascontiguousarray(xp[i])) for i in range(n)]
    res = run_bass_kernel_spmd(nc, in_maps, core_ids=list(range(n)))
    outs = [r["y"].reshape(C, H, W) for r in res.results]
    return np.stack(outs).astype(np.float32)

